# revision 18
# baseline (speedup 1.0000x reference)
"""Trainium2 Bass kernel for fused GEMM + row-LayerNorm + row-Softmax.

Computes, for x [M=65536, K=1024], weight [N=512, K], bias/gamma/beta [N],
scale [1]:
    acc  = x @ weight.T + bias
    norm = (acc - mean_row) / sqrt(var_row + EPS)
    out  = softmax(norm * gamma + beta) * scale, axis=-1)   (row softmax)

Sharding: data-parallel along M across 8 NeuronCores; weight/bias/gamma/
beta/scale replicated.  Host-side prep transposes x to [K, M] so each
k-chunk loads directly as the matmul's stationary operand (lhsT).

v3 layout: the PE does the pure GEMM only (8 accumulating matmuls per
128-row subtile; no rank-1 bias matmul).  The bias broadcast-add runs on
the DVE (the only engine that can read PSUM for tensor-tensor work),
writing f16 u to SBUF and releasing each PSUM bank right away instead of
holding it through the whole stats chain.  bn_stats/bn_aggr (DVE) and
exp (ACT, with free accum row-sum) read u; the final e*(1/sum) scale
runs on the otherwise-idle Pool/GpSimd engine.  Output is stored f16
(softmax probs; plenty of precision for rel-err 2e-2) and upcast to f32
on host, halving store traffic.  (A fused-stats path exists behind
BASS_STATS=fused — accum_out row-sums + one tensor_tensor_reduce for
sum-of-squares — but its ISA op crashes real HW, so default is bn.)
"""

import os

import numpy as np

import concourse.bass as bass
import concourse.tile as tile
from concourse import bacc, mybir
from concourse.bass_utils import run_bass_kernel_spmd

EPS = 1e-5
K = 1024
N = 512
M = 65536
N_CORES = 8
M_CORE = M // N_CORES
P = 128
KC = K // P  # k-chunks of 128
MSPAN = int(os.environ.get("BASS_MSPAN", "512"))  # m-cols per x block

F32 = mybir.dt.float32
F16 = mybir.dt.float16

# matmul input mode: "f16" (half storage+DMA, full-rate PE) is the default;
# "f32r"/"f32"/"bf16" kept for experiments.
MM_MODE = os.environ.get("BASS_MM_MODE", "f16")
# rstd path: "newton" = DVE-only rsqrt (fast-inverse-sqrt seed + 2 Newton
# steps) so Exp is the only ACT LUT function and the table loads exactly
# once; "lnexp" = ACT ln+exp fallback.
RSTD_MODE = os.environ.get("BASS_RSTD", "newton")
# stats path: "fused" = Su from the bias-add's accum_out + Su^2 via one
# f16 2x-rate tensor_tensor_reduce (no bn_stats/bn_aggr); "bn" = classic
# bn_stats+bn_aggr on u.
STATS_MODE = os.environ.get("BASS_STATS", "bn")
HB = int(os.environ.get("BASS_HB", "1"))  # subtiles per stats/exp group
OBLK_BUFS = int(os.environ.get("BASS_OBLK", "2"))
EOUT_BUFS = int(os.environ.get("BASS_EOUT", "7"))
UBUFS = int(os.environ.get("BASS_UBUFS", "8"))
# engine for the bias broadcast-add off the PE (reads PSUM: DVE only —
# walrus rejects GPSIMD PSUM access)
BIAS_ENG = os.environ.get("BASS_BIAS_ENG", "vector")
# engine for the final softmax scale e*(1/sum): "scalar" (ACT, per-
# partition scale via Identity) keeps it off the busier DVE
OMUL_ENG = os.environ.get("BASS_OMUL_ENG", "gpsimd")
# intermediate u = acc + bias dtype: f16 halves DVE/ACT read bandwidth;
# plenty of precision (u ~ +-3, rel 5e-4 << 2e-2 tolerance)
U_DT = os.environ.get("BASS_U_DT", "f16")
# issue output DMAs from this engine's HWDGE ring ("sync" or "scalar")
OUT_ENG = os.environ.get("BASS_OUT_ENG", "scalar")
XBUFS = int(os.environ.get("BASS_XBUFS", "3"))
# output store dtype: f16 halves store bytes; host upcasts to f32
OUT_DT = os.environ.get("BASS_OUT_DT", "f16")

_NC_CACHE: dict = {}


def _mm_dt(mode):
    return {
        "f32r": mybir.dt.float32r,
        "f32": F32,
        "f16": mybir.dt.float16,
        "bf16": mybir.dt.bfloat16,
    }[mode]


def _np_dt(mode):
    import ml_dtypes

    return {
        "f32r": np.float32,
        "f32": np.float32,
        "f16": np.float16,
        "bf16": ml_dtypes.bfloat16,
    }[mode]


def _out_dt():
    return F16 if OUT_DT == "f16" else F32


def _np_out_dt():
    return np.float16 if OUT_DT == "f16" else np.float32


def _build(m_core, mode, fast, gs_const, repeat=1):
    """Build + compile the per-core Bass module.

    fast: gamma*scale and beta*scale are constant across N -> fold the
    constant gamma*scale into rstd and drop the beta shift (softmax is
    invariant to per-row constants).  gs_const is that constant.

    repeat: run the whole pass `repeat` times back-to-back (same I/O) —
    used by the timing harness to measure the marginal cost of one pass.
    """
    x_dt = _mm_dt(mode)
    o_dt = _out_dt()
    nc = bacc.Bacc(
        "TRN2", target_bir_lowering=False, debug=False, num_devices=N_CORES
    )

    xt = nc.dram_tensor("xt", [K, m_core], x_dt, kind="ExternalInput").ap()
    wt = nc.dram_tensor("wt", [K, N], x_dt, kind="ExternalInput").ap()
    bias_d = nc.dram_tensor("bias", [N], F32, kind="ExternalInput").ap()
    if not fast:
        g2_d = nc.dram_tensor("gamma2", [N], F32, kind="ExternalInput").ap()
        b2_d = nc.dram_tensor("beta2", [N], F32, kind="ExternalInput").ap()
    out = nc.dram_tensor("out", [m_core, N], o_dt, kind="ExternalOutput").ap()

    xt_r = xt.rearrange("(c p) m -> c p m", p=P)
    wt_r = wt.rearrange("(c p) n -> c p n", p=P)

    def bcast(ap_1d, parts):
        return bass.AP(
            tensor=ap_1d.tensor, offset=ap_1d.offset, ap=[[0, parts]] + list(ap_1d.ap)
        )

    n_mblk = m_core // MSPAN
    n_sub = MSPAN // P

    with tile.TileContext(nc) as tc:
        with (
            tc.tile_pool(name="singles", bufs=1) as singles,
            tc.tile_pool(name="xin", bufs=XBUFS) as xin,
            tc.tile_pool(name="psum", bufs=8, space="PSUM") as ppool,
            tc.tile_pool(name="stats", bufs=8) as stats_p,
            tc.tile_pool(name="upool", bufs=UBUFS) as u_pool,
            tc.tile_pool(name="eout", bufs=EOUT_BUFS) as eout_p,
            tc.tile_pool(name="oblk", bufs=OBLK_BUFS) as oblk_p,
        ):
            # --- one-time setup ---
            # wt chunk DMAs are interleaved with the first x block's chunk
            # DMAs below so the first matmul waits on ~2 small DMAs, not
            # the whole 1MB weight load.
            wt_sb = singles.tile([P, KC, N], x_dt)
            # bias replicated across all 128 partitions for the bias add
            bias_rep = singles.tile([P, N], F32)
            nc.scalar.dma_start(out=bias_rep, in_=bcast(bias_d, P))
            eps_sb = singles.tile([P, 1], F32)
            nc.vector.memset(eps_sb, EPS)
            if not fast:
                g2b = singles.tile([P, N], F32)
                nc.sync.dma_start(out=g2b, in_=bcast(g2_d, P))
                b2b = singles.tile([P, N], F32)
                nc.sync.dma_start(out=b2b, in_=bcast(b2_d, P))

            gs = None if fast and gs_const == 1.0 else float(gs_const)
            out_r = out.rearrange("(b j p) n -> b j p n", j=n_sub, p=P)
            bias_eng = {
                "gpsimd": nc.gpsimd,
                "vector": nc.vector,
            }[BIAS_ENG]
            u_dt = F16 if (U_DT == "f16" and fast) else F32

            # --- main loop ---
            for rep in range(repeat):
                for ib in range(n_mblk):
                    ms = ib * MSPAN
                    x_tile = xin.tile([P, KC, MSPAN], x_dt, tag="x")
                    if rep == 0 and ib == 0:
                        # prologue: per-chunk DMAs, wt chunk c right before
                        # x chunk c, so matmul c waits only on its own pair
                        for c in range(KC):
                            nc.sync.dma_start(out=wt_sb[:, c, :], in_=wt_r[c])
                            nc.sync.dma_start(
                                out=x_tile[:, c, :],
                                in_=xt_r[c, :, ms : ms + MSPAN],
                            )
                    else:
                        nc.sync.dma_start(
                            out=x_tile,
                            in_=xt_r[:, :, ms : ms + MSPAN].rearrange(
                                "c p m -> p c m"
                            ),
                        )
                    o_blk = oblk_p.tile([P, n_sub, N], o_dt, tag="oblk")
                    for g in range(n_sub // HB):
                        # GEMM for HB subtiles; DVE adds bias into SBUF u
                        # (freeing each PSUM bank right away) and collects
                        # row-sum/row-sumsq; the batched rstd feeds the exps.
                        us = []
                        fused = (
                            STATS_MODE == "fused" and fast and RSTD_MODE == "newton"
                        )
                        if fused:
                            usumb = stats_p.tile([P, HB], F32, tag="usum")
                            ssqb = stats_p.tile([P, HB], F32, tag="ssq")
                        else:
                            mvb = stats_p.tile([P, HB, 2], F32, tag="mv")
                        for h in range(HB):
                            j = g * HB + h
                            acc = ppool.tile([P, N], F32, space="PSUM", tag="acc")
                            for c in range(KC):
                                nc.tensor.matmul(
                                    acc,
                                    x_tile[:, c, j * P : (j + 1) * P],
                                    wt_sb[:, c, :],
                                    start=(c == 0),
                                    stop=(c == KC - 1),
                                )
                            # u = acc + bias (PSUM read frees the bank
                            # without waiting on the whole stats chain);
                            # accum_out gives Su for free.
                            u = u_pool.tile([P, N], u_dt, tag="u")
                            bias_eng.scalar_tensor_tensor(
                                out=u,
                                in0=acc,
                                scalar=1.0,
                                in1=bias_rep,
                                op0=mybir.AluOpType.mult,
                                op1=mybir.AluOpType.add,
                                accum_out=usumb[:, h : h + 1] if fused else None,
                            )
                            us.append(u)
                            if fused:
                                # Su^2 (+ eps*N seed) in one f16 2x-rate op
                                u2 = u_pool.tile([P, N], u_dt, tag="u2")
                                nc.vector.tensor_tensor_reduce(
                                    out=u2,
                                    in0=u,
                                    in1=u,
                                    scale=1.0,
                                    scalar=float(EPS * N),
                                    op0=mybir.AluOpType.mult,
                                    op1=mybir.AluOpType.add,
                                    accum_out=ssqb[:, h : h + 1],
                                )
                            else:
                                st = stats_p.tile([P, 6], F32, tag="st")
                                nc.vector.bn_stats(out=st, in_=u)
                                nc.vector.bn_aggr(out=mvb[:, h, :], in_=st)

                        if fused:
                            # var+eps = ssq'/N - (usum/N)^2  (ssq' seeded
                            # with eps*N above)
                            tm = stats_p.tile([P, HB], F32, tag="tm")
                            nc.vector.tensor_mul(tm, usumb, usumb)
                            s1 = stats_p.tile([P, HB], F32, tag="s1")
                            nc.vector.tensor_scalar_mul(
                                out=s1, in0=ssqb, scalar1=1.0 / N
                            )
                            varb = stats_p.tile([P, HB], F32, tag="varb")
                            nc.vector.scalar_tensor_tensor(
                                out=varb,
                                in0=tm,
                                scalar=-1.0 / (N * N),
                                in1=s1,
                                op0=mybir.AluOpType.mult,
                                op1=mybir.AluOpType.add,
                            )
                        # Batched rstd = (var+eps)^-0.5 for the HB subtiles.
                        rstdb = stats_p.tile([P, HB], F32, tag="rstdb")
                        if RSTD_MODE == "newton":
                            # DVE-only rsqrt: fast-inverse-sqrt seed (float
                            # bits as a number: K - bits/2) + 2 Newton steps.
                            # Keeps Exp as the ONLY ACT function -> the ACT
                            # LUT loads once for the whole kernel.
                            I32 = mybir.dt.int32
                            if fused:
                                xe = varb
                            else:
                                xe = stats_p.tile([P, HB], F32, tag="xe")
                                nc.vector.tensor_scalar_add(
                                    out=xe, in0=mvb[:, :, 1], scalar1=EPS
                                )
                            bi = stats_p.tile([P, HB], F32, tag="bi")
                            nc.vector.tensor_copy(bi, xe.bitcast(I32))
                            y0f = stats_p.tile([P, HB], F32, tag="y0f")
                            nc.vector.tensor_scalar(
                                out=y0f,
                                in0=bi,
                                scalar1=-0.5,
                                scalar2=float(0x5F3759DF),
                                op0=mybir.AluOpType.mult,
                                op1=mybir.AluOpType.add,
                            )
                            y0i = stats_p.tile([P, HB], I32, tag="y0i")
                            nc.vector.tensor_copy(y0i, y0f)
                            y = y0i.bitcast(F32)
                            for it in range(2):
                                t = stats_p.tile([P, HB], F32, tag=f"nt{it}")
                                nc.vector.tensor_mul(t, xe, y)
                                nc.vector.tensor_mul(t, t, y)
                                nc.vector.tensor_scalar(
                                    out=t,
                                    in0=t,
                                    scalar1=-0.5,
                                    scalar2=1.5,
                                    op0=mybir.AluOpType.mult,
                                    op1=mybir.AluOpType.add,
                                )
                                dst = rstdb if it == 1 else stats_p.tile(
                                    [P, HB], F32, tag=f"ny{it}"
                                )
                                nc.vector.tensor_mul(dst, t, y)
                                y = dst
                        else:
                            # exp(-0.5*ln(var+eps)); Ln/Exp batched so the
                            # ACT LUT swaps twice per group, not per subtile
                            lnb = stats_p.tile([P, HB], F32, tag="lnb")
                            nc.scalar.activation(
                                out=lnb,
                                in_=mvb[:, :, 1],
                                func=mybir.ActivationFunctionType.Ln,
                                bias=eps_sb,
                            )
                            nc.scalar.activation(
                                out=rstdb,
                                in_=lnb,
                                func=mybir.ActivationFunctionType.Exp,
                                scale=-0.5,
                            )
                        if fast and gs is not None:
                            nc.vector.tensor_scalar_mul(
                                out=rstdb, in0=rstdb, scalar1=gs
                            )
                        # nmrb = -mean * rstd (per subtile column)
                        nmrb = stats_p.tile([P, HB], F32, tag="nmrb")
                        nc.vector.scalar_tensor_tensor(
                            out=nmrb,
                            in0=usumb if fused else mvb[:, :, 0],
                            scalar=(-1.0 / N) if fused else -1.0,
                            in1=rstdb,
                            op0=mybir.AluOpType.mult,
                            op1=mybir.AluOpType.mult,
                        )

                        ssumb = stats_p.tile([P, HB], F32, tag="ssumb")
                        e_ts = []
                        for h in range(HB):
                            j = g * HB + h
                            u = us[h]
                            e_t = eout_p.tile([P, N], u_dt, tag="e")
                            e_ts.append(e_t)
                            ssum = ssumb[:, h : h + 1]
                            if fast:
                                # e = exp(u*rstd - mean*rstd)
                                nc.scalar.activation(
                                    out=e_t,
                                    in_=u,
                                    func=mybir.ActivationFunctionType.Exp,
                                    bias=nmrb[:, h : h + 1],
                                    scale=rstdb[:, h : h + 1],
                                    accum_out=ssum,
                                )
                            else:
                                # z = ((u - mean) * gamma2) * rstd + beta2
                                u1 = eout_p.tile([P, N], F32, tag="u1")
                                nc.vector.scalar_tensor_tensor(
                                    out=u1,
                                    in0=u,
                                    scalar=mvb[:, h, 0:1],
                                    in1=g2b,
                                    op0=mybir.AluOpType.subtract,
                                    op1=mybir.AluOpType.mult,
                                )
                                z = eout_p.tile([P, N], F32, tag="z")
                                nc.vector.scalar_tensor_tensor(
                                    out=z,
                                    in0=u1,
                                    scalar=rstdb[:, h : h + 1],
                                    in1=b2b,
                                    op0=mybir.AluOpType.mult,
                                    op1=mybir.AluOpType.add,
                                )
                                nmax = stats_p.tile([P, 1], F32, tag="nmax")
                                nc.vector.tensor_reduce(
                                    out=nmax,
                                    in_=z,
                                    axis=mybir.AxisListType.X,
                                    op=mybir.AluOpType.max,
                                    negate=True,
                                )
                                nc.scalar.activation(
                                    out=e_t,
                                    in_=z,
                                    func=mybir.ActivationFunctionType.Exp,
                                    bias=nmax,
                                    accum_out=ssum,
                                )

                        # batched 1/sum for the group, then per-subtile scale
                        rdenb = stats_p.tile([P, HB], F32, tag="rdenb")
                        nc.vector.reciprocal(out=rdenb, in_=ssumb)
                        for h in range(HB):
                            j = g * HB + h
                            rden = rdenb[:, h : h + 1]
                            e_t = e_ts[h]
                            if OMUL_ENG == "scalar":
                                nc.scalar.mul(o_blk[:, j, :], e_t, rden)
                            elif OMUL_ENG == "gpsimd":
                                nc.gpsimd.tensor_scalar_mul(
                                    out=o_blk[:, j, :], in0=e_t, scalar1=rden
                                )
                            else:
                                nc.vector.tensor_scalar_mul(
                                    out=o_blk[:, j, :], in0=e_t, scalar1=rden
                                )
                    out_eng = {
                        "scalar": nc.scalar,
                        "sync": nc.sync,
                        "gpsimd": nc.gpsimd,
                    }[OUT_ENG]
                    out_eng.dma_start(
                        out=out_r[ib].rearrange("j p n -> p j n"),
                        in_=o_blk,
                    )

    nc.compile()
    return nc


def _get_nc(m_core, mode, fast, gs_const):
    key = (m_core, mode, fast, gs_const if fast else None)
    if key not in _NC_CACHE:
        _NC_CACHE[key] = _build(m_core, mode, fast, gs_const)
    return _NC_CACHE[key]


def _prep(x, weight, bias, gamma, beta, scale, mode):
    """Host-side prep shared by kernel() and the test harness."""
    np_dt = _np_dt(mode)
    s = float(np.asarray(scale).reshape(-1)[0])
    g2 = (np.asarray(gamma, np.float32) * s).astype(np.float32)
    b2 = (np.asarray(beta, np.float32) * s).astype(np.float32)
    fast = bool(np.all(g2 == g2[0]) and np.all(b2 == b2[0]))
    gs_const = float(g2[0]) if fast else 0.0
    xt = np.ascontiguousarray(np.asarray(x, np.float32).T).astype(np_dt)
    wt = np.ascontiguousarray(np.asarray(weight, np.float32).T).astype(np_dt)
    return xt, wt, np.asarray(bias, np.float32), g2, b2, fast, gs_const


def kernel(x, weight, bias, gamma, beta, scale):
    mode = MM_MODE
    xt, wt, bias_f, g2, b2, fast, gs_const = _prep(
        x, weight, bias, gamma, beta, scale, mode
    )
    m_core = x.shape[0] // N_CORES
    nc = _get_nc(m_core, mode, fast, gs_const)

    in_maps = []
    for c in range(N_CORES):
        im = {
            "xt": np.ascontiguousarray(xt[:, c * m_core : (c + 1) * m_core]),
            "wt": wt,
            "bias": bias_f,
        }
        if not fast:
            im["gamma2"] = g2
            im["beta2"] = b2
        in_maps.append(im)

    res = run_bass_kernel_spmd(nc, in_maps, list(range(N_CORES))).results
    out = np.concatenate([res[c]["out"] for c in range(N_CORES)], axis=0)
    return out.astype(np.float32, copy=False)
